# revision 1
# baseline (speedup 1.0000x reference)
"""Trainium2 Bass kernel for nn_CPCA (CPC-action loss).

Strategy: data-parallel over the env dim n (64 envs/core on 8 cores).
All heavy math on device in a feature-major ("transposed") layout:
  - 8-step GRU scan over action embeddings (f32r matmuls, fp32 state)
  - predictor MLP on positives (bf16/f32 mix)
  - 20 negatives per position gathered from the replicated rnn_inputs
    pool with int32 indirect DMAs (128 rows/call), transposed to
    feature-major via xbar DMA-transpose, then bf16 MLP.
Per-core partial sums (pos_loss_sum, neg_loss_sum, mask_sum) are
combined on the host into the scalar loss.

Measured on trn2 (8 cores): HW exec time ~567 us, relative error vs the
fp32 jax reference ~6.4e-5.
"""
import sys

if '/opt/trn_rl_repo' not in sys.path:
    sys.path.insert(0, '/opt/trn_rl_repo')

import numpy as np
import ml_dtypes

BF16 = ml_dtypes.bfloat16

N, T, H, TS, FS, K, A, ED, NNEG = 512, 128, 512, 6, 2, 8, 17, 32, 20
NCORE = 8
NE = N // NCORE          # 64 envs per core
P = NE * TS              # 384 positions per core
PF = FS * P              # 768 (f-major position columns)
NSLOT = PF * NNEG        # 15360 negative slots per core
CH = 640                 # gather-chunk slots (5*128, = 32 positions x 20)
NCHUNK = NSLOT // CH     # 24 gather chunks
NCALL = NSLOT // 128     # 120 indirect-gather calls (128 slots each)
CPC = CH // 128          # 5 gather calls per chunk
CHM = 320                # matmul sub-chunk slots (16 positions x 20)
NSUB = CH // CHM         # 2 matmul sub-chunks per gather chunk
CPOSM = CHM // NNEG      # 16 positions per matmul sub-chunk
HKC = H // 128           # 4

_PROG_CACHE = {}


# ----------------------------------------------------------------------------
# host-side input preparation (sharding / layout / index metadata only)
# ----------------------------------------------------------------------------

def _prep(inputs):
    acts = np.asarray(inputs['actions']).astype(np.int64)          # [N,T,1]
    nd = np.asarray(inputs['not_dones'], np.float32)               # [N,T,1]
    vld = np.asarray(inputs['valids']).astype(bool)                # [N,T,1]
    ri = np.asarray(inputs['rnn_inputs'], np.float32)              # [N,T,H]
    ro = np.asarray(inputs['rnn_outputs'], np.float32)             # [N,T,H]
    embw = np.asarray(inputs['embed_w'], np.float32)               # [A,ED]
    wih = np.asarray(inputs['gru_w_ih'], np.float32)               # [3H,ED]
    whh = np.asarray(inputs['gru_w_hh'], np.float32)               # [3H,H]
    bih = np.asarray(inputs['gru_b_ih'], np.float32)               # [3H]
    bhh = np.asarray(inputs['gru_b_hh'], np.float32)               # [3H]
    w1 = np.asarray(inputs['p_w1'], np.float32)                    # [H,2H]
    b1 = np.asarray(inputs['p_b1'], np.float32)                    # [H]
    w2 = np.asarray(inputs['p_w2'], np.float32)                    # [H,H]
    b2 = np.asarray(inputs['p_b2'], np.float32)                    # [H]
    w3 = np.asarray(inputs['p_w3'], np.float32)                    # [1,H]
    b3 = np.asarray(inputs['p_b3'], np.float32)                    # [1]
    tsub = np.asarray(inputs['time_subsample']).astype(np.int64)   # [TS]
    usub = np.asarray(inputs['unroll_subsample']).astype(np.int64) # [FS]
    negi = np.asarray(inputs['neg_indices']).astype(np.int64)      # [FS*TS*N*NNEG]
    maxk = int(np.asarray(inputs['max_k']))
    assert maxk == K, maxk
    assert tsub.shape == (TS,) and usub.shape == (FS,)

    # ---- shared (replicated) tensors -------------------------------------
    # GRU weights, transposed + chunked: dev[p, kc, g] = whh[g, kc*128+p]
    whh_dev = np.ascontiguousarray(
        whh.T.reshape(HKC, 128, 3 * H).transpose(1, 0, 2)).astype(BF16)  # [128,4,1536]
    wih_dev = np.zeros((128, 3 * H), np.float32)
    wih_dev[:ED] = wih.T                                            # zero-padded K
    wih_dev = wih_dev.astype(BF16)                                  # [128,1536] bf16

    def lhsT_chunks(w):  # w: [h_out=512, k=512] -> [128, 4, 512] (dev[p,kc,h]=w[h,kc*128+p])
        return np.ascontiguousarray(w.T.reshape(HKC, 128, H).transpose(1, 0, 2))

    w1a_dev = lhsT_chunks(w1[:, :H]).astype(BF16)
    w1b_dev = lhsT_chunks(w1[:, H:]).astype(BF16)
    w2_dev = lhsT_chunks(w2).astype(BF16)
    w3_dev = np.ascontiguousarray(w3[0].reshape(HKC, 128).T).astype(BF16)  # [128,4]

    brz_dev = np.ascontiguousarray((bih + bhh)[:2 * H].reshape(8, 128).T)  # [128,8]
    bhn_dev = np.ascontiguousarray(bhh[2 * H:].reshape(HKC, 128).T)        # [128,4]
    bin_dev = np.ascontiguousarray(bih[2 * H:].reshape(HKC, 128).T)        # [128,4]
    b1_dev = np.ascontiguousarray(b1.reshape(HKC, 128).T)
    b2_dev = np.ascontiguousarray(b2.reshape(HKC, 128).T)
    b3_dev = np.array([[b3[0], -b3[0]]], np.float32)                       # [1,2]

    # negatives pool (bf16), replicated to every core
    pool = np.ascontiguousarray(ri.reshape(N * T, H).astype(BF16))  # [65536,512]

    # one-hot band for the AT bias-add matmul: b4[p, c] = (c//NNEG == p%32)
    cc = np.arange(2 * CHM) // NNEG
    pp = np.arange(128) % 32
    b4 = (cc[None, :] == pp[:, None]).astype(BF16)                  # [128,640]

    # ---- per-core views ---------------------------------------------------
    ks = np.arange(K)
    tq = tsub[None, :] + ks[:, None]                                # [K,TS]
    ok_au = tq <= T - 2
    a_idx = acts[:, np.clip(tq, 0, T - 1), 0]                       # [N,K,TS]
    au_full = embw[a_idx] * ok_au[None, :, :, None]                 # [N,K,TS,ED]

    tf = tsub[None, :] + usub[:, None]                              # [FS,TS]
    ok_ft = tf <= T - 2
    ft_full = np.where(ok_ft[None, :, :, None],
                       ri[:, np.clip(tf + 1, 0, T - 1)], 0.0)       # [N,FS,TS,H]

    vm = ((nd[:, :, 0] > 0) & vld[:, :, 0]).astype(np.float32)      # [N,T]
    vmk = np.where(ok_au[None], vm[:, np.clip(tq, 0, T - 1)], 0.0)  # [N,K,TS]
    cum = np.cumprod(vmk, axis=1)                                   # [N,K,TS]
    maskf = cum[:, usub, :]                                         # [N,FS,TS]

    negi4 = negi.reshape(FS, N, TS, NNEG)

    in_maps = []
    for c in range(NCORE):
        sl = slice(c * NE, (c + 1) * NE)
        v = np.ascontiguousarray(negi4[:, sl]).reshape(-1)          # [15360]
        # idx32[p, j] = pool row for slot j*128+p
        idx32 = np.ascontiguousarray(
            v.astype(np.int32).reshape(NCALL, 128).T)               # [128,120]

        # h0: [128, 4, 384] dev[p,kc,j] = ro[i, ts_s, kc*128+p], j = il*6+s
        h0 = ro[sl][:, tsub].reshape(P, H).T                        # [H,P]
        ht0 = np.ascontiguousarray(h0.reshape(HKC, 128, P).transpose(1, 0, 2))

        # aut: [128, K, P] (zero-padded partitions ED..127)
        au_c = au_full[sl].transpose(1, 0, 2, 3).reshape(K, P, ED)  # [K,P,ED]
        aut = np.zeros((128, K, P), np.float32)
        aut[:ED] = au_c.transpose(2, 0, 1)
        aut = aut.astype(BF16)

        # ftt: [128, 4, 768] dev[p,kc,f*384+j] = ft[i, f, s, kc*128+p]
        ft_c = ft_full[sl].transpose(3, 1, 0, 2).reshape(H, FS * P) # [H, 768]
        ftt = np.ascontiguousarray(
            ft_c.reshape(HKC, 128, FS * P).transpose(1, 0, 2)).astype(BF16)

        msk = np.ascontiguousarray(
            maskf[sl].transpose(1, 0, 2).reshape(1, PF))            # [1,768]
        mskpos = msk[0]
        mskn = np.ascontiguousarray(
            np.repeat(mskpos, NNEG).reshape(128, NSLOT // 128)).astype(BF16)
        mskp = np.ascontiguousarray(mskpos.reshape(128, PF // 128)).astype(BF16)
        b3c = np.broadcast_to(np.array([b3[0], -b3[0]], np.float32),
                              (128, 2)).copy()

        in_maps.append(dict(
            whh=whh_dev, wih=wih_dev, aut=np.ascontiguousarray(aut),
            w1a=w1a_dev, w1b=w1b_dev, w2t=w2_dev, w3b=w3_dev,
            brz=brz_dev, bhn=bhn_dev, bin=bin_dev, b1t=b1_dev, b2t=b2_dev,
            b3v=b3_dev, ht0=ht0, ftt=ftt, mskt=msk,
            pool=pool, ix32=idx32, b4=b4,
            mskn=mskn, mskp=mskp, b3c=b3c,
        ))

    return in_maps, tuple(int(u) for u in usub)


# ----------------------------------------------------------------------------
# device program
# ----------------------------------------------------------------------------

def _build(usub_vals):
    import concourse.bass as bass
    from concourse.masks import make_identity
    import concourse.bacc as bacc
    import concourse.mybir as mybir
    import concourse.tile as tile

    dt = mybir.dt
    AF = mybir.ActivationFunctionType
    AL = mybir.AluOpType
    AX = mybir.AxisListType

    nc = bacc.Bacc("TRN2", target_bir_lowering=False, debug=False,
                   num_devices=NCORE)

    def din(name, shape, d):
        return nc.dram_tensor(name, shape, d, kind="ExternalInput").ap()

    whh = din("whh", [128, HKC, 3 * H], dt.bfloat16)
    wih = din("wih", [128, 3 * H], dt.bfloat16)
    aut = din("aut", [128, K, P], dt.bfloat16)
    w1a = din("w1a", [128, HKC, H], dt.bfloat16)
    w1b = din("w1b", [128, HKC, H], dt.bfloat16)
    w2t = din("w2t", [128, HKC, H], dt.bfloat16)
    w3b = din("w3b", [128, HKC], dt.bfloat16)
    brz = din("brz", [128, 8], dt.float32)
    bhn = din("bhn", [128, HKC], dt.float32)
    bin_ = din("bin", [128, HKC], dt.float32)
    b1t = din("b1t", [128, HKC], dt.float32)
    b2t = din("b2t", [128, HKC], dt.float32)
    b3v = din("b3v", [1, 2], dt.float32)
    ht0 = din("ht0", [128, HKC, P], dt.float32)
    ftt = din("ftt", [128, HKC, PF], dt.bfloat16)
    mskt = din("mskt", [1, PF], dt.float32)
    poold = din("pool", [N * T, H], dt.bfloat16)
    ix32 = din("ix32", [128, NCALL], dt.int32)
    b4d = din("b4", [128, 2 * CHM], dt.bfloat16)
    msknd = din("mskn", [128, NSLOT // 128], dt.bfloat16)
    mskpd = din("mskp", [128, PF // 128], dt.bfloat16)
    b3cd = din("b3c", [128, 2], dt.float32)
    out = nc.dram_tensor("out", [1, 4], dt.float32, kind="ExternalOutput").ap()

    with tile.TileContext(nc) as tc:
        with (
            tc.tile_pool(name="cw", bufs=1) as cw,
            tc.tile_pool(name="ps", bufs=6, space="PSUM") as ps,
            tc.tile_pool(name="pst", bufs=2, space="PSUM") as pst,
            tc.tile_pool(name="gp", bufs=2) as gp,
            tc.tile_pool(name="ng", bufs=2) as ng,
        ):
            def load(name, ap_, shape, d):
                t = cw.tile(shape, d, tag=name, name=name)
                nc.sync.dma_start(out=t[:], in_=ap_[:])
                return t

            tWHH = cw.tile([128, HKC, 3 * H], dt.bfloat16, tag="whh",
                           name="whh")
            for _kc in range(HKC):
                nc.sync.dma_start(out=tWHH[:, _kc, :], in_=whh[:, _kc, :])
            tWIH = load("wih", wih, [128, 3 * H], dt.bfloat16)
            tAUT = load("aut", aut, [128, K, P], dt.bfloat16)
            tW1A = load("w1a", w1a, [128, HKC, H], dt.bfloat16)
            tW1B = load("w1b", w1b, [128, HKC, H], dt.bfloat16)
            tW2 = load("w2t", w2t, [128, HKC, H], dt.bfloat16)
            tW3 = load("w3b", w3b, [128, HKC], dt.bfloat16)
            tBRZ = load("brz", brz, [128, 8], dt.float32)
            tBHN = load("bhn", bhn, [128, HKC], dt.float32)
            tBIN = load("bin", bin_, [128, HKC], dt.float32)
            tB1 = load("b1t", b1t, [128, HKC], dt.float32)
            tB2 = load("b2t", b2t, [128, HKC], dt.float32)
            tB3 = load("b3v", b3v, [1, 2], dt.float32)
            tFTT = load("ftt", ftt, [128, HKC, PF], dt.bfloat16)
            tMSK = load("mskt", mskt, [1, PF], dt.float32)
            tIX = load("ix32", ix32, [128, NCALL], dt.int32)
            tB4 = load("b4", b4d, [128, 2 * CHM], dt.bfloat16)
            tMSKN = load("mskn", msknd, [128, NSLOT // 128], dt.bfloat16)
            tMSKP = load("mskp", mskpd, [128, PF // 128], dt.bfloat16)
            tB3C = load("b3c", b3cd, [128, 2], dt.float32)

            tHT = [cw.tile([128, HKC, P], dt.float32, tag=f"ht{i}",
                           name=f"ht{i}")
                   for i in range(2)]
            nc.sync.dma_start(out=tHT[0][:], in_=ht0[:])

            tHTB = [cw.tile([128, HKC, P], dt.bfloat16, tag=f"htb{i}",
                            name=f"htb{i}")
                    for i in range(2)]
            nc.vector.tensor_copy(out=tHTB[0][:], in_=tHT[0][:])
            tFPT = cw.tile([128, HKC, PF], dt.bfloat16, tag="fpt")
            tAT = cw.tile([128, HKC, PF], dt.float32, tag="at")
            tR = cw.tile([128, HKC, P], dt.float32, tag="r")
            tZ = cw.tile([128, HKC, P], dt.float32, tag="z")
            tLOGN = cw.tile([1, NSLOT], dt.bfloat16, tag="logn")
            tLOGP = cw.tile([1, PF], dt.bfloat16, tag="logp")
            tRES = cw.tile([1, 4], dt.float32, tag="res")
            tID = cw.tile([128, 128], dt.bfloat16, tag="ident", name="ident")
            make_identity(nc, tID[:])
            tIDF = cw.tile([128, 128], dt.float32, tag="identf", name="identf")
            make_identity(nc, tIDF[:])
            tATT = cw.tile([128, PF // 128, H], dt.bfloat16, tag="att",
                           name="att")

            gc_tiles = {}

            def produce_gc(ct):
                gc = ng.tile([128, HKC, CH], dt.bfloat16, tag="gc",
                             name=f"gc{ct}", bufs=7)
                for jj in range(CPC):
                    j = ct * CPC + jj
                    gr = ng.tile([128, H], dt.bfloat16, tag="gr", name="gr",
                                 bufs=8)
                    nc.gpsimd.indirect_dma_start(
                        out=gr[:], out_offset=None, in_=poold[:],
                        in_offset=bass.IndirectOffsetOnAxis(
                            ap=tIX[:, j:j + 1], axis=0))
                    pt = pst.tile([128, 512], dt.bfloat16, tag="pt", name="pt")
                    for b in range(HKC):
                        nc.tensor.transpose(
                            out=pt[:, b * 128:(b + 1) * 128],
                            in_=gr[:, b * 128:(b + 1) * 128],
                            identity=tID[:])
                    nc.vector.tensor_copy(
                        out=gc[:, :, jj * 128:(jj + 1) * 128],
                        in_=pt[:].rearrange("p (b c) -> p b c", c=128))
                gc_tiles[ct] = gc

            tLV = cw.tile([128, NSLOT // 128], dt.bfloat16, tag="lv",
                          name="lv")
            tLPV = cw.tile([128, PF // 128], dt.bfloat16, tag="lpv",
                           name="lpv")
            tAN = cw.tile([128, 2], dt.float32, tag="an", name="an")
            tONE = cw.tile([128, 1], dt.float32, tag="one", name="one")
            nc.vector.memset(tONE[:], 1.0)
            with tc.tile_pool(name="dsc", bufs=1, space="DRAM") as dsc:
                dLOG = dsc.tile([1, NSLOT + PF], dt.bfloat16, name="dlog")

                NCF = NCHUNK // FS   # chunks per unroll index
                PREFETCH = 3

                def emit_f_section(f):
                    """generator: yields between work pieces so the GRU loop
                    can interleave emission (PE gap filler)."""
                    cols = slice(f * P, (f + 1) * P)
                    # AT = W1a @ fp + b1 for this half
                    for ht in range(HKC):
                        p = ps.tile([128, 512], dt.float32, tag="ps", name="p")
                        for kc in range(HKC):
                            nc.tensor.matmul(
                                p[:, :P],
                                lhsT=tW1A[:, kc, ht * 128:(ht + 1) * 128],
                                rhs=tFPT[:, kc, cols],
                                start=(kc == 0), stop=(kc == HKC - 1))
                        nc.scalar.activation(
                            out=tAT[:, ht, cols], in_=p[:, :P],
                            func=AF.Identity, bias=tB1[:, ht:ht + 1])
                    # ATT[pos, blk, h] = AT^T for the one-hot bias matmuls
                    for ht in range(HKC):
                        for pb in range(f * (P // 128), (f + 1) * (P // 128)):
                            pat_ = pst.tile([128, 128], dt.float32, tag="pt",
                                            name="pat")
                            nc.tensor.transpose(
                                out=pat_[:],
                                in_=tAT[:, ht, pb * 128:(pb + 1) * 128],
                                identity=tIDF[:])
                            nc.vector.tensor_copy(
                                out=tATT[:, pb, ht * 128:(ht + 1) * 128],
                                in_=pat_[:])
                    # positives half
                    h1 = ng.tile([128, HKC, P], dt.bfloat16, tag="h1n",
                                 name="h1p", bufs=3)
                    for ht in range(HKC):
                        p = ps.tile([128, 512], dt.float32, tag="ps", name="p")
                        for kc in range(HKC):
                            nc.tensor.matmul(
                                p[:, :P],
                                lhsT=tW1B[:, kc, ht * 128:(ht + 1) * 128],
                                rhs=tFTT[:, kc, cols],
                                start=(kc == 0), stop=(kc == HKC - 1))
                        nc.vector.tensor_add(
                            out=p[:, :P], in0=p[:, :P], in1=tAT[:, ht, cols])
                        nc.scalar.activation(
                            out=h1[:, ht, :], in_=p[:, :P], func=AF.Relu)
                    h2 = ng.tile([128, HKC, P], dt.bfloat16, tag="h2n",
                                 name="h2p", bufs=3)
                    for ht in range(HKC):
                        p = ps.tile([128, 512], dt.float32, tag="ps", name="p")
                        for kc in range(HKC):
                            nc.tensor.matmul(
                                p[:, :P],
                                lhsT=tW2[:, kc, ht * 128:(ht + 1) * 128],
                                rhs=h1[:, kc, :],
                                start=(kc == 0), stop=(kc == HKC - 1))
                        nc.scalar.activation(
                            out=h2[:, ht, :], in_=p[:, :P], func=AF.Relu,
                            bias=tB2[:, ht:ht + 1])
                    pl = ps.tile([1, 512], dt.float32, tag="ps", name="pl")
                    for kc in range(HKC):
                        nc.tensor.matmul(
                            pl[:, :P], lhsT=tW3[:, kc:kc + 1], rhs=h2[:, kc, :],
                            start=(kc == 0), stop=(kc == HKC - 1))
                    nc.vector.tensor_copy(out=tLOGP[0:1, cols], in_=pl[:, :P])
                    yield
                    # negatives chunks for this half
                    ct0 = f * NCF
                    for ci in range(min(PREFETCH, NCF)):
                        produce_gc(ct0 + ci)
                    for ci in range(NCF):
                        ct = ct0 + ci
                        if ci + PREFETCH < NCF:
                            produce_gc(ct + PREFETCH)
                        gc = gc_tiles[ct]
                        for m in range(NSUB):
                            cm = ct * NSUB + m
                            mcols = slice(m * CHM, (m + 1) * CHM)
                            win = (cm * CPOSM // 32) * 32
                            off = cm * CPOSM - win
                            base = win % 128
                            blk = win // 128
                            h1 = ng.tile([128, HKC, CHM], dt.bfloat16,
                                         tag="h1n", name="h1", bufs=3)
                            for ht in range(HKC):
                                p = ps.tile([128, 512], dt.float32, tag="ps",
                                            name="p")
                                for kc in range(HKC):
                                    nc.tensor.matmul(
                                        p[:, :CHM],
                                        lhsT=tW1B[:, kc,
                                                  ht * 128:(ht + 1) * 128],
                                        rhs=gc[:, kc, mcols],
                                        start=(kc == 0), stop=False)
                                nc.tensor.matmul(
                                    p[:, :CHM],
                                    lhsT=tATT[base:base + 32, blk,
                                              ht * 128:(ht + 1) * 128],
                                    rhs=tB4[base:base + 32,
                                            off * NNEG:off * NNEG + CHM],
                                    start=False, stop=True,
                                    tile_position=(base, 0))
                                nc.scalar.activation(
                                    out=h1[:, ht, :], in_=p[:, :CHM],
                                    func=AF.Relu)
                            h2 = ng.tile([128, HKC, CHM], dt.bfloat16,
                                         tag="h2n", name="h2", bufs=3)
                            for ht in range(HKC):
                                p = ps.tile([128, 512], dt.float32, tag="ps",
                                            name="p")
                                for kc in range(HKC):
                                    nc.tensor.matmul(
                                        p[:, :CHM],
                                        lhsT=tW2[:, kc,
                                                 ht * 128:(ht + 1) * 128],
                                        rhs=h1[:, kc, :],
                                        start=(kc == 0), stop=(kc == HKC - 1))
                                nc.scalar.activation(
                                    out=h2[:, ht, :], in_=p[:, :CHM],
                                    func=AF.Relu, bias=tB2[:, ht:ht + 1])
                            pl = ps.tile([1, 512], dt.float32, tag="ps",
                                         name="pl")
                            for kc in range(HKC):
                                nc.tensor.matmul(
                                    pl[:, :CHM], lhsT=tW3[:, kc:kc + 1],
                                    rhs=h2[:, kc, :],
                                    start=(kc == 0), stop=(kc == HKC - 1))
                            nc.vector.tensor_copy(
                                out=tLOGN[0:1, cm * CHM:(cm + 1) * CHM],
                                in_=pl[:, :CHM])
                        yield

                # ---------------- GRU scan + per-f sections ----------------
                forder = sorted(range(FS), key=lambda f: (usub_vals[f], f))
                pending = []
                for k in range(K):
                    cur, nxt = tHT[k % 2], tHT[(k + 1) % 2]
                    curb = tHTB[k % 2]
                    for gt in range(8):
                        p = ps.tile([128, 512], dt.float32, tag="ps")
                        for kc in range(HKC):
                            nc.tensor.matmul(
                                p[:, :P],
                                lhsT=tWHH[:, kc, gt * 128:(gt + 1) * 128],
                                rhs=curb[:, kc, :],
                                start=(kc == 0), stop=False)
                        nc.tensor.matmul(
                            p[:, :P],
                            lhsT=tWIH[:, gt * 128:(gt + 1) * 128],
                            rhs=tAUT[:, k, :],
                            start=False, stop=True)
                        dst = tR if gt < 4 else tZ
                        nc.scalar.activation(
                            out=dst[:, gt % 4, :], in_=p[:, :P],
                            func=AF.Sigmoid, bias=tBRZ[:, gt:gt + 1])
                    for ct in range(HKC):
                        gt = 8 + ct
                        ph = ps.tile([128, 512], dt.float32, tag="ps")
                        for kc in range(HKC):
                            nc.tensor.matmul(
                                ph[:, :P],
                                lhsT=tWHH[:, kc, gt * 128:(gt + 1) * 128],
                                rhs=curb[:, kc, :],
                                start=(kc == 0), stop=(kc == HKC - 1))
                        pi = ps.tile([128, 512], dt.float32, tag="ps")
                        nc.tensor.matmul(
                            pi[:, :P],
                            lhsT=tWIH[:, gt * 128:(gt + 1) * 128],
                            rhs=tAUT[:, k, :],
                            start=True, stop=True)
                        t1_ = gp.tile([128, P], dt.float32, tag="t1")
                        nc.vector.scalar_tensor_tensor(
                            out=t1_[:], in0=ph[:, :P],
                            scalar=tBHN[:, ct:ct + 1],
                            in1=tR[:, ct, :], op0=AL.add, op1=AL.mult)
                        nc.vector.tensor_add(out=t1_[:], in0=t1_[:],
                                             in1=pi[:, :P])
                        tc_ = gp.tile([128, P], dt.float32, tag="tc")
                        nc.scalar.activation(
                            out=tc_[:], in_=t1_[:], func=AF.Tanh,
                            bias=tBIN[:, ct:ct + 1])
                        t2_ = gp.tile([128, P], dt.float32, tag="t2")
                        nc.vector.tensor_sub(out=t2_[:], in0=cur[:, ct, :],
                                             in1=tc_[:])
                        nc.vector.tensor_mul(out=t2_[:], in0=t2_[:],
                                             in1=tZ[:, ct, :])
                        nc.vector.tensor_add(out=nxt[:, ct, :], in0=t2_[:],
                                             in1=tc_[:])
                        nc.vector.tensor_copy(out=tHTB[(k + 1) % 2][:, ct, :],
                                              in_=nxt[:, ct, :])
                    for f in forder:
                        if usub_vals[f] == k:
                            for _kc in range(HKC):
                                nc.vector.tensor_copy(
                                    out=tFPT[:, _kc, f * P:(f + 1) * P],
                                    in_=nxt[:, _kc, :])
                            pending.append(emit_f_section(f))
                    # interleave a few pieces of ready sections between steps
                    pulls = 4 if k < K - 1 else None
                    while pending and (pulls is None or pulls > 0):
                        try:
                            next(pending[0])
                            if pulls is not None:
                                pulls -= 1
                        except StopIteration:
                            pending.pop(0)


                # ---------------- final partials ----------------
                # spread logits across 128 partitions via DRAM bounce
                nc.sync.dma_start(out=dLOG[0:1, :NSLOT], in_=tLOGN[:])
                nc.sync.dma_start(out=dLOG[0:1, NSLOT:], in_=tLOGP[:])
                nc.sync.dma_start(
                    out=tLV[:],
                    in_=dLOG[0:1, :NSLOT].rearrange("a (p c) -> (a p) c",
                                                    p=128))
                nc.sync.dma_start(
                    out=tLPV[:],
                    in_=dLOG[0:1, NSLOT:].rearrange("a (p c) -> (a p) c",
                                                    p=128))
                # neg: sum(mask * softplus(x+b3)) = sum(ln(1 + mask*exp(x+b3)))
                nc.scalar.activation(out=tLV[:], in_=tLV[:], func=AF.Exp,
                                     bias=tB3C[:, 0:1])
                nc.vector.tensor_mul(out=tLV[:], in0=tLV[:], in1=tMSKN[:])
                nc.scalar.activation(out=tLV[:], in_=tLV[:], func=AF.Ln,
                                     bias=1.0, accum_out=tAN[:, 1:2])
                # pos: sum(mask * softplus(-(x+b3)))
                nc.scalar.activation(out=tLPV[:], in_=tLPV[:], func=AF.Exp,
                                     scale=-1.0, bias=tB3C[:, 1:2])
                nc.vector.tensor_mul(out=tLPV[:], in0=tLPV[:], in1=tMSKP[:])
                nc.scalar.activation(out=tLPV[:], in_=tLPV[:], func=AF.Ln,
                                     bias=1.0, accum_out=tAN[:, 0:1])
                # partition-reduce the two accумulators
                for col in range(2):
                    pr = ps.tile([1, 512], dt.float32, tag="ps", name="pr")
                    nc.tensor.matmul(pr[:, :1], lhsT=tAN[:, col:col + 1],
                                     rhs=tONE[:], start=True, stop=True)
                    nc.vector.tensor_copy(out=tRES[0:1, col:col + 1],
                                          in_=pr[:, :1])
                nc.vector.tensor_reduce(tRES[0:1, 2:3], tMSK[:], AX.X, AL.add)
                nc.vector.memset(tRES[0:1, 3:4], 0.0)
                nc.sync.dma_start(out=out[:], in_=tRES[:])

    nc.compile()
    return nc


def _get_program(usub_vals):
    key = usub_vals
    if key not in _PROG_CACHE:
        _PROG_CACHE[key] = _build(usub_vals)
    return _PROG_CACHE[key]


def kernel(**inputs):
    from concourse.bass_utils import run_bass_kernel_spmd
    in_maps, usub_vals = _prep(inputs)
    nc = _get_program(usub_vals)
    res = run_bass_kernel_spmd(nc, in_maps, list(range(NCORE)))
    parts = np.stack([np.asarray(res.results[c]['out'][0], np.float64)
                      for c in range(NCORE)])
    pos, neg, den = parts[:, 0].sum(), parts[:, 1].sum(), parts[:, 2].sum()
    return np.float32(0.1 * (pos / den + neg / (den * NNEG)))



# revision 14
# speedup vs baseline: 1.5374x; 1.5374x over previous
"""Trainium2 Bass kernel for nn_CPCA (CPC-action loss).

Strategy: data-parallel over the env dim n (64 envs/core on 8 cores).
v3:
  - fp8 DoubleRow matmuls (2x PE rate) for the GRU and both MLP paths.
  - negatives fetched by 120 single-column indirect DMAs (128 fp8 rows
    each) and transposed on the PE at u16 granularity into the
    pair-packed layout DoubleRow consumes (logical k = 256*ktile +
    2*partition + byte); PSUM->SBUF copies alternate DVE/Scalar.
    Gather+transpose emission is paced against the consumption rate so
    the PE queue never blocks on a not-yet-gathered chunk.
  - GRU biases folded into a constant-1 row of the padded action
    embeddings; zero k-tile plane interleaved host-side so no per-step
    copies are needed.
  - layer-1 PSUM extraction fused into one custom DVE op
    relu(in0 + in1) with the shared per-position term broadcast via a
    stride-0 AP; layer-2 extraction fused into Scalar activations
    (bias+relu+fp8 cast).
  - single 32KB DRAM logit bounce at the tail; mask denominator is
    summed on the host.
Per-core partial sums (pos_loss_sum, neg_loss_sum) are combined with
the host-side mask count into the scalar loss.
"""
import sys

if '/opt/trn_rl_repo' not in sys.path:
    sys.path.insert(0, '/opt/trn_rl_repo')

import numpy as np
import ml_dtypes

BF16 = ml_dtypes.bfloat16
FP8 = ml_dtypes.float8_e4m3   # IEEE e4m3 (max 240) == TRN fp8_exp4

N, T, H, TS, FS, K, A, ED, NNEG = 512, 128, 512, 6, 2, 8, 17, 32, 20
NCORE = 8
NE = N // NCORE          # 64 envs per core
P = NE * TS              # 384 positions per core (per unroll index)
PF = FS * P              # 768
NSLOT = FS * P * NNEG    # 15360 negative slots per core
NCALL = NSLOT // 128     # 120 indirect gather calls (128 rows each)
SC = 24 * NNEG           # 480 slots (24 positions) per matmul sub-chunk
NSC = NSLOT // SC        # 32 sub-chunks (16 per unroll index)
SCF = NSC // FS          # 16
HKC = H // 128           # 4
POOL_BIAS = 32768

_PROG_CACHE = {}
USE_CUSTOM_DVE = True
USE_DMA_GATHER = True


# ----------------------------------------------------------------------------
# custom DVE op: out = relu(in0 + in1)   (in1 may be a stride-0 broadcast)
# ----------------------------------------------------------------------------

def _relu_add_op():
    from concourse import dve_ops
    from concourse.dve_spec import Spec, Src0, Src1, relu, lower
    from concourse.dve_uop import DveOpSpec

    name = "RELU_ADD_CPCA"
    for op in dve_ops.OPS:
        if op.name == name:
            return op

    def _ref(in0, in1, c0, c1, c2):
        x = np.asarray(in0, np.float32) + \
            np.asarray(in1, np.float32).reshape(np.asarray(in0).shape)
        return np.maximum(
            np.nan_to_num(x, nan=0.0, posinf=np.inf, neginf=-np.inf), 0)

    spec = Spec(body=relu(Src0 + Src1), reference=_ref)
    shas = {}
    for ver in ("v3", "v4"):
        tmp = DveOpSpec(name=name, opcode=31, uops=lower(spec, ver=ver),
                        rd1_en=True)
        shas[ver] = tmp.sha(ver)
    op = dve_ops.DveOp(name, spec, subdim=False, uops_sha=shas)
    dve_ops.OPS.append(op)
    dve_ops.CUSTOM_DVE_SPECS[name] = spec
    dve_ops._SUB_OPCODE_FOR_NAME[name] = (
        dve_ops._CUSTOM_DVE_ROW_BASE + len(dve_ops.OPS) - 1)
    assert dve_ops._SUB_OPCODE_FOR_NAME[name] < 0x20
    return op


# ----------------------------------------------------------------------------
# host-side input preparation (sharding / layout / index metadata only)
# ----------------------------------------------------------------------------

def _prep(inputs):
    acts = np.asarray(inputs['actions']).astype(np.int64)          # [N,T,1]
    nd = np.asarray(inputs['not_dones'], np.float32)               # [N,T,1]
    vld = np.asarray(inputs['valids']).astype(bool)                # [N,T,1]
    ri = np.asarray(inputs['rnn_inputs'], np.float32)              # [N,T,H]
    ro = np.asarray(inputs['rnn_outputs'], np.float32)             # [N,T,H]
    embw = np.asarray(inputs['embed_w'], np.float32)               # [A,ED]
    wih = np.asarray(inputs['gru_w_ih'], np.float32)               # [3H,ED]
    whh = np.asarray(inputs['gru_w_hh'], np.float32)               # [3H,H]
    bih = np.asarray(inputs['gru_b_ih'], np.float32)               # [3H]
    bhh = np.asarray(inputs['gru_b_hh'], np.float32)               # [3H]
    w1 = np.asarray(inputs['p_w1'], np.float32)                    # [H,2H]
    b1 = np.asarray(inputs['p_b1'], np.float32)                    # [H]
    w2 = np.asarray(inputs['p_w2'], np.float32)                    # [H,H]
    b2 = np.asarray(inputs['p_b2'], np.float32)                    # [H]
    w3 = np.asarray(inputs['p_w3'], np.float32)                    # [1,H]
    b3 = np.asarray(inputs['p_b3'], np.float32)                    # [1]
    tsub = np.asarray(inputs['time_subsample']).astype(np.int64)   # [TS]
    usub = np.asarray(inputs['unroll_subsample']).astype(np.int64) # [FS]
    negi = np.asarray(inputs['neg_indices']).astype(np.int64)      # [FS*TS*N*NNEG]
    maxk = int(np.asarray(inputs['max_k']))
    assert maxk == K, maxk
    assert tsub.shape == (TS,) and usub.shape == (FS,)

    forder = np.argsort(usub, kind='stable')                       # consumption order

    # ---- shared (replicated) tensors -------------------------------------
    def dr_std(w):
        # [p, g, i, m] = w[m, (2g+i)*128 + p]
        return np.ascontiguousarray(
            w.T.reshape(2, 2, 128, -1).transpose(2, 0, 1, 3)).astype(FP8)

    def dr_pair(w):
        # [p, g, i, m] = w[m, 256g + 2p + i] (matches gather u16-pair layout)
        return np.ascontiguousarray(
            w.T.reshape(2, 128, 2, -1).transpose(1, 0, 2, 3)).astype(FP8)

    # GRU combined lhsT: k-tiles 0-3 = whh.T chunks, 4 = wih.T (+bias row 32),
    # 5 = zeros.  Gates r/z get bih+bhh; n gets bih only (bhh n-part is
    # applied pre-multiplied by r via the stt scalar).
    wk = np.zeros((6, 128, 3 * H), np.float32)
    wk[:4] = whh.T.reshape(4, 128, 3 * H)
    wk[4, :ED] = wih.T
    wk[4, ED] = np.concatenate([(bih + bhh)[:2 * H], bih[2 * H:]])
    whhc = np.ascontiguousarray(
        wk.reshape(3, 2, 128, 3 * H).transpose(2, 0, 1, 3)).astype(FP8)

    w1ap = dr_std(w1[:, :H])
    w1bp = dr_std(w1[:, H:])
    w1bq = dr_pair(w1[:, H:])
    w2p = dr_std(w2)
    # layer-3 weights broadcast to all 128 output rows — skinny (M<4) DR
    # ldweights fail the walrus ISA check; only PSUM partition 0 is read.
    w3p = np.ascontiguousarray(np.broadcast_to(
        w3[0].reshape(2, 2, 128).transpose(2, 0, 1)[..., None],
        (128, 2, 2, 128))).astype(FP8)

    bhn_dev = np.ascontiguousarray(bhh[2 * H:].reshape(HKC, 128).T)  # [128,4]
    b1_dev = np.ascontiguousarray(b1.reshape(HKC, 128).T)
    b2_dev = np.ascontiguousarray(b2.reshape(HKC, 128).T)
    b3c = np.broadcast_to(np.array([b3[0], -b3[0]], np.float32), (128, 2)).copy()

    # negatives pool: fp8, u16-pair rows for the transpose-gather
    pool8 = ri.reshape(N * T, H).astype(FP8)
    pool_u16 = np.ascontiguousarray(pool8).view(BF16)               # [65536,256]

    # ---- per-core views ---------------------------------------------------
    ks = np.arange(K)
    tq = tsub[None, :] + ks[:, None]                                # [K,TS]
    ok_au = tq <= T - 2
    a_idx = acts[:, np.clip(tq, 0, T - 1), 0]                       # [N,K,TS]
    au_full = embw[a_idx] * ok_au[None, :, :, None]                 # [N,K,TS,ED]

    tf = tsub[None, :] + usub[:, None]                              # [FS,TS]
    ok_ft = tf <= T - 2
    ft_full = np.where(ok_ft[None, :, :, None],
                       ri[:, np.clip(tf + 1, 0, T - 1)], 0.0)       # [N,FS,TS,H]

    vm = ((nd[:, :, 0] > 0) & vld[:, :, 0]).astype(np.float32)      # [N,T]
    vmk = np.where(ok_au[None], vm[:, np.clip(tq, 0, T - 1)], 0.0)  # [N,K,TS]
    cum = np.cumprod(vmk, axis=1)                                   # [N,K,TS]
    maskf = cum[:, usub, :]                                         # [N,FS,TS]

    negi4 = negi.reshape(FS, N, TS, NNEG)

    in_maps = []
    denoms = []
    for c in range(NCORE):
        sl = slice(c * NE, (c + 1) * NE)

        # h0: [128, 4, P] dev[p,kc,j] = ro[i, ts_s, kc*128+p], j = i*TS+s
        h0 = ro[sl][:, tsub].reshape(P, H).T                        # [H,P]
        ht0 = np.ascontiguousarray(h0.reshape(HKC, 128, P).transpose(1, 0, 2))
        ht0b = ht0.astype(BF16)
        ht08 = ht0.astype(FP8)

        # aut2: [128, K, 2, P]: plane 0 = action embedding rows 0-31 +
        # constant-1 bias row 32; plane 1 = zeros (DoubleRow zero k-tile)
        au_c = au_full[sl].transpose(1, 0, 2, 3).reshape(K, P, ED)  # [K,P,ED]
        aut2 = np.zeros((128, K, 2, P), np.float32)
        aut2[:ED, :, 0, :] = au_c.transpose(2, 0, 1)
        aut2[ED, :, 0, :] = 1.0
        aut2 = aut2.astype(FP8)

        # ftt: [128, 4, PF] in consumption (fi) order
        ft_c = ft_full[sl][:, forder].transpose(3, 1, 0, 2).reshape(H, PF)
        ftt = np.ascontiguousarray(
            ft_c.reshape(HKC, 128, PF).transpose(1, 0, 2)).astype(FP8)

        # masks, fi-ordered position flat index = fi*P + i*TS + s
        posflat = np.ascontiguousarray(
            maskf[sl][:, forder].transpose(1, 0, 2)).reshape(PF)    # [768]
        negflat = np.repeat(posflat, NNEG)                          # [15360]
        mskp = np.ascontiguousarray(posflat.reshape(128, PF // 128)).astype(BF16)
        mskn = np.ascontiguousarray(negflat.reshape(128, NSLOT // 128)).astype(BF16)
        denoms.append(float(posflat.sum()))

        # negative indices, fi-ordered: ix32[p, c] = slot c*128+p
        v = np.concatenate([negi4[f, sl].reshape(-1) for f in forder])
        ix32 = np.ascontiguousarray(
            v.astype(np.int32).reshape(NCALL, 128).T)

        in_maps.append(dict(
            whhc=whhc, aut2=np.ascontiguousarray(aut2),
            ht0b=ht0b, ht08=ht08,
            w1ap=w1ap, w1bp=w1bp, w1bq=w1bq, w2p=w2p, w3p=w3p,
            bhn=bhn_dev, b1t=b1_dev, b2t=b2_dev, b3c=b3c,
            ftt=ftt, pool=pool_u16, ix32=ix32, mskn=mskn, mskp=mskp,
        ))

    return in_maps, tuple(int(u) for u in usub), sum(denoms)


# ----------------------------------------------------------------------------
# device program
# ----------------------------------------------------------------------------

def _build(usub_vals):
    import concourse.bass as bass
    import concourse.bacc as bacc
    import concourse.mybir as mybir
    import concourse.tile as tile

    dt = mybir.dt
    AF = mybir.ActivationFunctionType
    AL = mybir.AluOpType
    DR = mybir.MatmulPerfMode.DoubleRow
    RELU_ADD = _relu_add_op()

    forder = sorted(range(FS), key=lambda f: (usub_vals[f], f))

    nc = bacc.Bacc("TRN2", target_bir_lowering=False, debug=False,
                   num_devices=NCORE)

    def din(name, shape, d):
        return nc.dram_tensor(name, shape, d, kind="ExternalInput").ap()

    whhc = din("whhc", [128, 3, 2, 3 * H], dt.float8e4)
    aut2 = din("aut2", [128, K, 2, P], dt.float8e4)
    ht0b = din("ht0b", [128, HKC, P], dt.bfloat16)
    ht08 = din("ht08", [128, HKC, P], dt.float8e4)
    w1ap = din("w1ap", [128, 2, 2, H], dt.float8e4)
    w1bp = din("w1bp", [128, 2, 2, H], dt.float8e4)
    w1bq = din("w1bq", [128, 2, 2, H], dt.float8e4)
    w2p = din("w2p", [128, 2, 2, H], dt.float8e4)
    w3p = din("w3p", [128, 2, 2, 128], dt.float8e4)
    bhn = din("bhn", [128, HKC], dt.float32)
    b1t = din("b1t", [128, HKC], dt.float32)
    b2t = din("b2t", [128, HKC], dt.float32)
    b3c = din("b3c", [128, 2], dt.float32)
    ftt = din("ftt", [128, HKC, PF], dt.float8e4)
    poold = din("pool", [N * T, H // 2], dt.bfloat16)
    ixd = din("ix32", [128, NCALL], dt.int32)
    msknd = din("mskn", [128, NSLOT // 128], dt.bfloat16)
    mskpd = din("mskp", [128, PF // 128], dt.bfloat16)
    out = nc.dram_tensor("out", [1, 4], dt.float32, kind="ExternalOutput").ap()

    with tile.TileContext(nc) as tc:
        with (
            tc.tile_pool(name="cw", bufs=1) as cw,
            tc.tile_pool(name="ps2", bufs=3, space="PSUM") as ps2,
            tc.tile_pool(name="plp", bufs=1, space="PSUM") as plp,
            tc.tile_pool(name="pst", bufs=1, space="PSUM") as pst,
            tc.tile_pool(name="ng", bufs=3) as ng,
            tc.tile_pool(name="grp", bufs=8) as grp,
        ):
            def load(name, ap_, shape, d):
                t = cw.tile(shape, d, tag=name, name=name)
                nc.sync.dma_start(out=t[:], in_=ap_[:])
                return t

            # gather indices + GRU-critical loads first
            tIX = load("ix32", ixd, [128, NCALL], dt.int32)
            tWHH = cw.tile([128, 3, 2, 3 * H], dt.float8e4, tag="whhc",
                           name="whhc")
            for g in range(3):
                nc.sync.dma_start(out=tWHH[:, g], in_=whhc[:, g])
            tAUT = load("aut2", aut2, [128, K, 2, P], dt.float8e4)
            tHT = [cw.tile([128, HKC, P], dt.bfloat16, tag=f"ht{i}",
                           name=f"ht{i}") for i in range(2)]
            nc.sync.dma_start(out=tHT[0][:], in_=ht0b[:])
            tC8 = [cw.tile([128, HKC, P], dt.float8e4, tag=f"c8{i}",
                           name=f"c8{i}") for i in range(2)]
            nc.sync.dma_start(out=tC8[0][:], in_=ht08[:])

            tW1A = load("w1ap", w1ap, [128, 2, 2, H], dt.float8e4)
            tW1B = load("w1bp", w1bp, [128, 2, 2, H], dt.float8e4)
            tW1Q = load("w1bq", w1bq, [128, 2, 2, H], dt.float8e4)
            tW2 = load("w2p", w2p, [128, 2, 2, H], dt.float8e4)
            tW3 = load("w3p", w3p, [128, 2, 2, 128], dt.float8e4)
            tBHN = load("bhn", bhn, [128, HKC], dt.float32)
            tB1 = load("b1t", b1t, [128, HKC], dt.float32)
            tB2 = load("b2t", b2t, [128, HKC], dt.float32)
            tB3C = load("b3c", b3c, [128, 2], dt.float32)
            tFTT = load("ftt", ftt, [128, HKC, PF], dt.float8e4)
            tMSKN = load("mskn", msknd, [128, NSLOT // 128], dt.bfloat16)
            tMSKP = load("mskp", mskpd, [128, PF // 128], dt.bfloat16)

            # persistent state tiles
            tAT = cw.tile([128, HKC, PF], dt.bfloat16, tag="at", name="at")
            tR = cw.tile([128, HKC, P], dt.bfloat16, tag="r", name="r")
            tZ = cw.tile([128, HKC, P], dt.bfloat16, tag="z", name="z")
            tGC = cw.tile([128, 2, NSLOT], dt.bfloat16, tag="gc", name="gc")
            tROWN = cw.tile([1, NSLOT], dt.bfloat16, tag="rown", name="rown")
            tROWP = cw.tile([1, PF], dt.bfloat16, tag="rowp", name="rowp")
            tLV = cw.tile([128, NSLOT // 128], dt.bfloat16, tag="lv", name="lv")
            tLPV = cw.tile([128, PF // 128], dt.bfloat16, tag="lpv", name="lpv")
            tAN = cw.tile([128, 2], dt.float32, tag="an", name="an")
            tONE = cw.tile([128, 1], dt.float32, tag="one", name="one")
            nc.vector.memset(tONE[:], 1.0)
            tRES = cw.tile([1, 4], dt.float32, tag="res", name="res")

            tIDU = cw.tile([128, 128], dt.bfloat16, tag="idu", name="idu")
            from concourse.masks import make_identity
            make_identity(nc, tIDU[:])

            # gather + PE u16-pair transpose + copy, paced by ensure_calls
            _calls = [0]

            def emit_call():
                g = _calls[0]
                _calls[0] += 1
                gr = grp.tile([128, H // 2], dt.bfloat16, tag="gr",
                              name=f"gr{g}")
                nc.gpsimd.indirect_dma_start(
                    out=gr[:], out_offset=None, in_=poold[:],
                    in_offset=bass.IndirectOffsetOnAxis(
                        ap=tIX[:, g:g + 1], axis=0))
                pt = pst.tile([128, 2, 128], dt.bfloat16, tag="pt", name="pt")
                for b in range(2):
                    nc.tensor.transpose(
                        out=pt[:, b, :], in_=gr[:, b * 128:(b + 1) * 128],
                        identity=tIDU[:])
                eng = nc.vector if g % 2 == 0 else nc.scalar
                if eng is nc.vector:
                    nc.vector.tensor_copy(
                        out=tGC[:, :, g * 128:(g + 1) * 128], in_=pt[:])
                else:
                    nc.scalar.activation(
                        out=tGC[:, :, g * 128:(g + 1) * 128], in_=pt[:],
                        func=AF.Identity)

            def ensure_calls(n):
                while _calls[0] < min(n, NCALL):
                    emit_call()

            # fp8 view of the gathered pool: [p][ktile i][slot]
            def gc_rhs(g, cm, w):
                c0 = cm * SC
                return tGC[:, g, :].bitcast(dt.float8e4).rearrange(
                    "p (s i) -> p i s", i=2)[:, :, c0:c0 + w]

            # ---------------- per-f section (generator) ----------------
            def emit_f_section(fi, n8):
                cols = slice(fi * P, (fi + 1) * P)
                # AT = W1a @ fp + b1  (fp = n8)
                for hp in range(2):
                    p2 = ps2.tile([128, 2, 512], dt.float32, tag="ps")
                    for j in range(2):
                        ht = hp * 2 + j
                        for g in range(2):
                            nc.tensor.matmul(
                                p2[:, j, :P],
                                lhsT=tW1A[:, g, :, ht * 128:(ht + 1) * 128],
                                rhs=n8[:, 2 * g:2 * g + 2, :],
                                start=(g == 0), stop=(g == 1), perf_mode=DR)
                    for j in range(2):
                        ht = hp * 2 + j
                        nc.scalar.activation(
                            out=tAT[:, ht, cols], in_=p2[:, j, :P],
                            func=AF.Identity, bias=tB1[:, ht:ht + 1])
                yield
                # positives: h1 = relu(W1b@ft + AT); h2 = relu(W2@h1+b2)
                h1 = ng.tile([128, HKC, P], dt.float8e4, tag="h1", name="h1p")
                for hp in range(2):
                    p2 = ps2.tile([128, 2, 512], dt.float32, tag="ps")
                    for j in range(2):
                        ht = hp * 2 + j
                        for g in range(2):
                            nc.tensor.matmul(
                                p2[:, j, :P],
                                lhsT=tW1B[:, g, :, ht * 128:(ht + 1) * 128],
                                rhs=tFTT[:, 2 * g:2 * g + 2, cols],
                                start=(g == 0), stop=(g == 1), perf_mode=DR)
                    for j in range(2):
                        ht = hp * 2 + j
                        if USE_CUSTOM_DVE:
                            nc.vector._custom_dve(
                                RELU_ADD, out=h1[:, ht, :], in0=p2[:, j, :P],
                                in1=tAT[:, ht, cols])
                        else:
                            nc.vector.tensor_add(
                                out=p2[:, j, :P], in0=p2[:, j, :P],
                                in1=tAT[:, ht, cols])
                            nc.scalar.activation(
                                out=h1[:, ht, :], in_=p2[:, j, :P],
                                func=AF.Relu)
                yield
                h2 = ng.tile([128, HKC, P], dt.float8e4, tag="h2", name="h2p")
                for hp in range(2):
                    p2 = ps2.tile([128, 2, 512], dt.float32, tag="ps")
                    for j in range(2):
                        ht = hp * 2 + j
                        for g in range(2):
                            nc.tensor.matmul(
                                p2[:, j, :P],
                                lhsT=tW2[:, g, :, ht * 128:(ht + 1) * 128],
                                rhs=h1[:, 2 * g:2 * g + 2, :],
                                start=(g == 0), stop=(g == 1), perf_mode=DR)
                    for j in range(2):
                        ht = hp * 2 + j
                        nc.scalar.activation(
                            out=h2[:, ht, :], in_=p2[:, j, :P],
                            func=AF.Relu, bias=tB2[:, ht:ht + 1])
                pl = plp.tile([128, 512], dt.float32, tag="pl")
                for g in range(2):
                    nc.tensor.matmul(
                        pl[:, :P], lhsT=tW3[:, g], rhs=h2[:, 2 * g:2 * g + 2, :],
                        start=(g == 0), stop=(g == 1), perf_mode=DR)
                nc.scalar.activation(out=tROWP[0:1, fi * P:(fi + 1) * P],
                                     in_=pl[0:1, :P], func=AF.Identity)
                yield
                # negatives: 16 sub-chunks of 480 slots (24 positions)
                for m in range(SCF):
                    cm = fi * SCF + m
                    c0 = cm * SC
                    a0 = fi * P + m * 24
                    ensure_calls((((cm + 1) * SC + 127) // 128) + 8)
                    h1n = ng.tile([128, HKC, SC], dt.float8e4, tag="h1",
                                  name="h1n")
                    for hp in range(2):
                        p2 = ps2.tile([128, 2, 512], dt.float32, tag="ps")
                        for j in range(2):
                            ht = hp * 2 + j
                            for g in range(2):
                                nc.tensor.matmul(
                                    p2[:, j, :SC],
                                    lhsT=tW1Q[:, g, :, ht * 128:(ht + 1) * 128],
                                    rhs=gc_rhs(g, cm, SC),
                                    start=(g == 0), stop=(g == 1), perf_mode=DR)
                        for j in range(2):
                            ht = hp * 2 + j
                            if USE_CUSTOM_DVE:
                                nc.vector._custom_dve(
                                    RELU_ADD,
                                    out=h1n[:, ht, :].rearrange(
                                        "p (a b) -> p a b", b=NNEG),
                                    in0=p2[:, j, :SC].rearrange(
                                        "p (a b) -> p a b", b=NNEG),
                                    in1=tAT[:, ht, a0:a0 + 24][:, :, None]
                                    .broadcast_to((128, 24, NNEG)))
                            else:
                                nc.vector.tensor_add(
                                    out=p2[:, j, :SC].rearrange(
                                        "p (a b) -> p a b", b=NNEG),
                                    in0=p2[:, j, :SC].rearrange(
                                        "p (a b) -> p a b", b=NNEG),
                                    in1=tAT[:, ht, a0:a0 + 24][:, :, None]
                                    .broadcast_to((128, 24, NNEG)))
                                nc.scalar.activation(
                                    out=h1n[:, ht, :], in_=p2[:, j, :SC],
                                    func=AF.Relu)
                    h2n = ng.tile([128, HKC, SC], dt.float8e4, tag="h2",
                                  name="h2n")
                    for hp in range(2):
                        p2 = ps2.tile([128, 2, 512], dt.float32, tag="ps")
                        for j in range(2):
                            ht = hp * 2 + j
                            for g in range(2):
                                nc.tensor.matmul(
                                    p2[:, j, :SC],
                                    lhsT=tW2[:, g, :, ht * 128:(ht + 1) * 128],
                                    rhs=h1n[:, 2 * g:2 * g + 2, :],
                                    start=(g == 0), stop=(g == 1), perf_mode=DR)
                        for j in range(2):
                            ht = hp * 2 + j
                            nc.scalar.activation(
                                out=h2n[:, ht, :], in_=p2[:, j, :SC],
                                func=AF.Relu, bias=tB2[:, ht:ht + 1])
                    pl = plp.tile([128, 512], dt.float32, tag="pl")
                    for g in range(2):
                        nc.tensor.matmul(
                            pl[:, :SC], lhsT=tW3[:, g],
                            rhs=h2n[:, 2 * g:2 * g + 2, :],
                            start=(g == 0), stop=(g == 1), perf_mode=DR)
                    if cm % 2 == 0:
                        nc.vector.tensor_copy(out=tROWN[0:1, c0:c0 + SC],
                                              in_=pl[0:1, :SC])
                    else:
                        nc.scalar.activation(out=tROWN[0:1, c0:c0 + SC],
                                             in_=pl[0:1, :SC], func=AF.Identity)
                    yield

            # ---------------- GRU scan + interleaving ----------------
            pending = []
            for k in range(K):
                c8, n8 = tC8[k % 2], tC8[(k + 1) % 2]
                hcur, hnxt = tHT[k % 2], tHT[(k + 1) % 2]
                # r (gates 0-3) and z (gates 4-7), batched in ct pairs
                for gh in range(4):
                    gbase = (gh // 2) * 4 + (gh % 2) * 2
                    p2 = ps2.tile([128, 2, 512], dt.float32, tag="ps")
                    for j in range(2):
                        gt = gbase + j
                        for g in range(3):
                            rhs = (c8[:, 2 * g:2 * g + 2, :] if g < 2
                                   else tAUT[:, k])
                            nc.tensor.matmul(
                                p2[:, j, :P],
                                lhsT=tWHH[:, g, :, gt * 128:(gt + 1) * 128],
                                rhs=rhs,
                                start=(g == 0), stop=(g == 2), perf_mode=DR)
                    dst = tR if gh < 2 else tZ
                    cp = (gh % 2) * 2
                    nc.scalar.activation(
                        out=dst[:, cp:cp + 2, :], in_=p2[:, :, :P],
                        func=AF.Sigmoid)
                # n gates + state update, in ct pairs
                for cp in range(2):
                    ph2 = ps2.tile([128, 2, 512], dt.float32, tag="ps")
                    pi2 = ps2.tile([128, 2, 512], dt.float32, tag="ps")
                    for j in range(2):
                        ct = cp * 2 + j
                        gt = 8 + ct
                        for g in range(2):
                            nc.tensor.matmul(
                                ph2[:, j, :P],
                                lhsT=tWHH[:, g, :, gt * 128:(gt + 1) * 128],
                                rhs=c8[:, 2 * g:2 * g + 2, :],
                                start=(g == 0), stop=(g == 1), perf_mode=DR)
                        nc.tensor.matmul(
                            pi2[:, j, :P],
                            lhsT=tWHH[:, 2, :, gt * 128:(gt + 1) * 128],
                            rhs=tAUT[:, k],
                            start=True, stop=True, perf_mode=DR)
                    t2 = ng.tile([128, 2, P], dt.bfloat16, tag="tm", name="t2")
                    for j in range(2):
                        ct = cp * 2 + j
                        nc.vector.scalar_tensor_tensor(
                            out=t2[:, j, :], in0=ph2[:, j, :P],
                            scalar=tBHN[:, ct:ct + 1], in1=tR[:, ct, :],
                            op0=AL.add, op1=AL.mult)
                    nc.vector.tensor_add(out=t2[:], in0=t2[:],
                                         in1=pi2[:, :, :P])
                    c2 = ng.tile([128, 2, P], dt.bfloat16, tag="tm", name="c2")
                    nc.scalar.activation(out=c2[:], in_=t2[:], func=AF.Tanh)
                    sl2 = slice(cp * 2, cp * 2 + 2)
                    d2 = ng.tile([128, 2, P], dt.bfloat16, tag="tm", name="d2")
                    nc.vector.tensor_sub(out=d2[:], in0=hcur[:, sl2, :],
                                         in1=c2[:])
                    nc.vector.tensor_mul(out=d2[:], in0=d2[:],
                                         in1=tZ[:, sl2, :])
                    nc.vector.tensor_add(out=hnxt[:, sl2, :], in0=d2[:],
                                         in1=c2[:])
                    nc.scalar.activation(out=n8[:, sl2, :],
                                         in_=hnxt[:, sl2, :], func=AF.Identity)
                ensure_calls(5 * (k + 1))
                for fi in range(FS):
                    if usub_vals[forder[fi]] == k:
                        pending.append(emit_f_section(fi, n8))
                pulls = 2 if k < K - 1 else None
                while pending and (pulls is None or pulls > 0):
                    try:
                        next(pending[0])
                        if pulls is not None:
                            pulls -= 1
                    except StopIteration:
                        pending.pop(0)

            ensure_calls(NCALL)
            # ---------------- final partials ----------------
            with tc.tile_pool(name="dsc", bufs=1, space="DRAM") as dsc:
                dROW = dsc.tile([1, NSLOT + PF], dt.bfloat16, name="drow")
                nc.sync.dma_start(out=dROW[0:1, :NSLOT], in_=tROWN[:])
                nc.sync.dma_start(out=dROW[0:1, NSLOT:], in_=tROWP[:])
                nc.sync.dma_start(
                    out=tLV[:],
                    in_=dROW[0:1, :NSLOT].rearrange("a (p c) -> (a p) c",
                                                    p=128))
                nc.sync.dma_start(
                    out=tLPV[:],
                    in_=dROW[0:1, NSLOT:].rearrange("a (p c) -> (a p) c",
                                                    p=128))
            # neg: sum(mask * softplus(x+b3)) = sum(ln(1 + mask*exp(x+b3)))
            nc.scalar.activation(out=tLV[:], in_=tLV[:], func=AF.Exp,
                                 bias=tB3C[:, 0:1])
            nc.vector.tensor_mul(out=tLV[:], in0=tLV[:], in1=tMSKN[:])
            nc.scalar.activation(out=tLV[:], in_=tLV[:], func=AF.Ln,
                                 bias=1.0, accum_out=tAN[:, 1:2])
            # pos: sum(mask * softplus(-(x+b3)))
            nc.scalar.activation(out=tLPV[:], in_=tLPV[:], func=AF.Exp,
                                 scale=-1.0, bias=tB3C[:, 1:2])
            nc.vector.tensor_mul(out=tLPV[:], in0=tLPV[:], in1=tMSKP[:])
            nc.scalar.activation(out=tLPV[:], in_=tLPV[:], func=AF.Ln,
                                 bias=1.0, accum_out=tAN[:, 0:1])
            for col in range(2):
                pr = plp.tile([128, 512], dt.float32, tag="pl", name="pr")
                nc.tensor.matmul(pr[:1, :1], lhsT=tAN[:, col:col + 1],
                                 rhs=tONE[:], start=True, stop=True)
                nc.vector.tensor_copy(out=tRES[0:1, col:col + 1],
                                      in_=pr[:1, :1])
            nc.vector.memset(tRES[0:1, 2:4], 0.0)
            nc.sync.dma_start(out=out[:], in_=tRES[:])

    nc.compile()
    return nc


def _get_program(usub_vals):
    key = usub_vals
    if key not in _PROG_CACHE:
        _PROG_CACHE[key] = _build(usub_vals)
    return _PROG_CACHE[key]


def kernel(**inputs):
    from concourse.bass_utils import run_bass_kernel_spmd
    in_maps, usub_vals, denom = _prep(inputs)
    nc = _get_program(usub_vals)
    res = run_bass_kernel_spmd(nc, in_maps, list(range(NCORE)))
    parts = np.stack([np.asarray(res.results[c]['out'][0], np.float64)
                      for c in range(NCORE)])
    pos, neg = parts[:, 0].sum(), parts[:, 1].sum()
    return np.float32(0.1 * (pos / denom + neg / (denom * NNEG)))
